# revision 24
# baseline (speedup 1.0000x reference)
"""Trainium2 Bass kernel for nn_CurvedMultiHeadAttention (B=4, S=1024, E=768, H=12, D=64, R=16).

Sharding: 8 cores; core c handles batch b=c//2 and heads h0=6*(c%2) .. h0+5
(head-parallel within a batch element). Each core computes a partial output
(its 6 heads' out-projection contribution, with bo/2 folded in); the host sums
the two partials per batch element (the unshard step for head sharding).

Math restructuring (validated against the reference at ~1e-6 rel err):
 - softmax over keys is invariant to per-query additive shifts => the qq term
   of the Mahalanobis distance drops entirely.
 - the EPS*I part of G_h contributes <1e-5 relative to scores => dropped.
 - scoresT[t,s] = sum_r kAT[r,t]*qAT[r,s];  per-key bias ckk[t] = -SCALE*kk[t]
   + mask[t] is applied as the ScalarE activation bias during exp (scoresT has
   keys on partitions, so the bias axis is the partition axis).
 - qA = (hidden @ Wq^T + bq) @ A is computed as hidden @ (Wq^T A) + bq A:
   Weff = A^T Wq is built on-device with tiny matmuls, so the big projection
   emits 16 (padded to 32) output dims per head instead of 64.
 - softmax denominator comes free as a ones column appended to v in the ctx
   matmul; bo/2 is added via a GpSimd-broadcast row during PSUM evacuation.

All heavy matmuls are bf16 with fp32 PSUM accumulation (measured end-to-end
rel err ~2.4e-3). The score/exp stage of head h+1 is software-pipelined with
the ctx stage of head h so ScalarE (exp) and TensorE overlap.
"""

import os
import numpy as np
import ml_dtypes

import concourse.bass as bass
import concourse.tile as tile
from concourse import bacc
from concourse import mybir
from concourse.bass_utils import run_bass_kernel_spmd
from concourse.masks import make_identity

F32 = mybir.dt.float32
BF16 = mybir.dt.bfloat16
AF = mybir.ActivationFunctionType

S = 1024          # sequence length
E = 768           # embed
D = 64            # head dim
R = 16            # rank
HPC = 6           # heads per core
NCORES = 8
SCALE = 1.0 / 8.0
ESC = 2.0 * SCALE  # exp scale

EAUG = E + 1            # 769 (ones row folds biases into the projections)
KCH = [128] * 6 + [1]   # contraction chunking of EAUG
WEFFW = 32 * HPC        # Weff columns, 32 per head (16 real + 16 pad)

LAST_RESULTS = None     # BassKernelResults of the most recent run (for test.py)


def _emit(tc):
    nc = tc.nc
    hTa = nc.dram_tensor("hTa", [EAUG, S], BF16, kind="ExternalInput")
    wqr = nc.dram_tensor("Wqr", [HPC * D, EAUG], BF16, kind="ExternalInput")
    wkr = nc.dram_tensor("Wkr", [HPC * D, EAUG], BF16, kind="ExternalInput")
    wvd = nc.dram_tensor("WvTa", [EAUG, HPC * D], BF16, kind="ExternalInput")
    wod = nc.dram_tensor("WoT", [HPC * D, E], BF16, kind="ExternalInput")
    apd = nc.dram_tensor("Apack", [D, WEFFW], BF16, kind="ExternalInput")
    mkd = nc.dram_tensor("maskT", [128, S // 128], F32, kind="ExternalInput")
    bod = nc.dram_tensor("bo2", [1, E], F32, kind="ExternalInput")
    outd = nc.dram_tensor("outp", [S, E], F32, kind="ExternalOutput")

    import contextlib
    stack = contextlib.ExitStack()
    const = stack.enter_context(tc.tile_pool(name="const", bufs=1))
    work = stack.enter_context(tc.tile_pool(name="work", bufs=4))
    ptp = stack.enter_context(tc.tile_pool(name="ptp", bufs=16))
    psp = stack.enter_context(tc.tile_pool(name="psp", bufs=3, space="PSUM"))

    def psum():
        return psp.tile([128, 1024], F32, name="ps", tag="ps")

    def psum_small():
        return psp.tile([128, 128], F32, name="psx", tag="pst", bufs=2)

    def psum_bf():
        return psp.tile([128, 128], BF16, name="pst", tag="pst", bufs=2)

    dma = nc.sync.dma_start
    cp = nc.vector.tensor_copy

    # ---------------- constant / weight loads ----------------
    hT, wv = [], []
    for i, kc in enumerate(KCH):
        r0 = 128 * i
        hT.append(const.tile([kc, S], BF16, name=f"hT{i}", tag=f"hT{i}"))
        dma(out=hT[i][:, :], in_=hTa[r0:r0 + kc, :])
        wv.append(const.tile([kc, HPC * D], BF16, name=f"wv{i}", tag=f"wv{i}"))
        dma(out=wv[i][:, :], in_=wvd[r0:r0 + kc, :])
    wqh, wkh = [], []
    for h in range(HPC):
        wqh.append(const.tile([D, EAUG], BF16, name=f"wqh{h}", tag=f"wqh{h}"))
        dma(out=wqh[h][:, :], in_=wqr[D * h:D * (h + 1), :])
        wkh.append(const.tile([D, EAUG], BF16, name=f"wkh{h}", tag=f"wkh{h}"))
        dma(out=wkh[h][:, :], in_=wkr[D * h:D * (h + 1), :])
    wo = []
    for i in range(3):
        wo.append(const.tile([128, E], BF16, name=f"wo{i}", tag=f"wo{i}"))
        dma(out=wo[i][:, :], in_=wod[128 * i:128 * (i + 1), :])
    apk = const.tile([D, WEFFW], BF16, name="apk", tag="apk")
    dma(out=apk[:, :], in_=apd[:, :])
    maskT = const.tile([128, S // 128], F32, name="maskT", tag="maskT")
    dma(out=maskT[:, :], in_=mkd[:, :])
    bo2 = const.tile([1, E], F32, name="bo2", tag="bo2")
    dma(out=bo2[:, :], in_=bod[:, :])
    bo_bc = const.tile([128, E], F32, name="bo_bc", tag="bo_bc")
    nc.gpsimd.partition_broadcast(bo_bc[:, :], bo2[:, :])

    ones16 = const.tile([R, 1], BF16, name="ones16", tag="ones16")
    nc.vector.memset(ones16[:, :], 1.0)
    ident = const.tile([128, 128], BF16, name="ident", tag="ident")
    make_identity(nc, ident[:, :])

    vsb = [const.tile([128, HPC * (D + 1)], BF16, name=f"v{t}", tag=f"v{t}") for t in range(8)]
    ctxn = [const.tile([128, HPC * D], BF16, name=f"ctxn{s}", tag=f"ctxn{s}") for s in range(8)]
    ctxT = [const.tile([128, S], BF16, name=f"ctxT{j}", tag=f"ctxT{j}") for j in range(3)]

    # ---------------- v projection -> vsb (bf16, ones col interleaved) --------
    for t in range(8):
        pv = psum()
        for k in range(7):
            nc.tensor.matmul(
                out=pv[:, 0:HPC * D],
                lhsT=hT[k][:, 128 * t:128 * (t + 1)],
                rhs=wv[k][:, :],
                start=(k == 0), stop=(k == 6),
            )
        vst = work.tile([128, HPC * D], BF16, name="vst", tag="vst", bufs=2)
        cp(vst[:, :], pv[:, 0:HPC * D])
        vv = vsb[t][:, :].rearrange("p (h c) -> p h c", h=HPC)   # (128, 6, 65)
        cp(vv[:, :, 0:D], vst[:, :].rearrange("p (h c) -> p h c", h=HPC))
        nc.vector.memset(vv[:, :, D:D + 1], 1.0)

    # ---------------- Weff = [A^T Wq ; A^T bq] on device ----------------
    # weff{q,k}[ec] : (128|1, 192) bf16, rows = e (769 total), col 32h+r
    weff = {}
    for key, wh in (("q", wqh), ("k", wkh)):
        tiles = []
        for ec, kc in enumerate(KCH):
            pw = psum()
            for h in range(HPC):
                nc.tensor.matmul(
                    out=pw[0:kc, 32 * h:32 * h + R],
                    lhsT=wh[h][:, 128 * ec:128 * ec + kc],
                    rhs=apk[:, 32 * h:32 * h + R],
                    start=True, stop=True,
                )
            wt = const.tile([kc, WEFFW], BF16, name=f"weff{key}{ec}",
                            tag=f"weff{key}{ec}")
            cp(wt[:, :], pw[0:kc, 0:WEFFW])
            tiles.append(wt)
        weff[key] = tiles

    # ---------------- qAT/kAT for all heads: Weff.T @ hTa ----------------
    # two partition groups: heads 0-3 (cols 0:128) and heads 4-5 (cols 128:192)
    qk = {"q": [], "k": []}
    for key in ("q", "k"):
        for mt, mp in ((0, 128), (1, 64)):
            pq = psum()
            for n in range(2):
                for k in range(7):
                    nc.tensor.matmul(
                        out=pq[0:mp, 512 * n:512 * (n + 1)],
                        lhsT=weff[key][k][:, 128 * mt:128 * mt + mp],
                        rhs=hT[k][:, 512 * n:512 * (n + 1)],
                        start=(k == 0), stop=(k == 6),
                    )
            # per-head copies to base-0 (16, S) bf16 tiles
            for hh in range(4 if mt == 0 else 2):
                th = work.tile([R, S], BF16, name=f"{key}a", tag=f"{key}a", bufs=7)
                cp(th[:, :], pq[32 * hh:32 * hh + R, :])
                qk[key].append(th)

    # ---------------- per-head attention, software-pipelined ----------------
    def stage_a(h):
        """scores + exp for head h; returns the 8 PT tiles."""
        qa, ka = qk["q"][h], qk["k"][h]
        ksq = work.tile([R, S], BF16, name="ksq", tag="ksq", bufs=2)
        nc.vector.tensor_mul(ksq[:, :], ka[:, :], ka[:, :])
        pk = psum()
        for t in range(8):
            nc.tensor.matmul(
                out=pk[:, t:t + 1],
                lhsT=ksq[:, 128 * t:128 * (t + 1)],
                rhs=ones16[:, :],
                start=True, stop=True,
            )
        ckkT = work.tile([128, S // 128], F32, name="ckkT", tag="ckkT", bufs=2)
        nc.vector.scalar_tensor_tensor(
            out=ckkT[:, :], in0=pk[:, 0:S // 128], scalar=-SCALE,
            in1=maskT[:, :], op0=mybir.AluOpType.mult, op1=mybir.AluOpType.add,
        )
        pts = []
        for t in range(8):
            pc = psum()
            for n in range(2):
                nc.tensor.matmul(
                    out=pc[:, 512 * n:512 * (n + 1)],
                    lhsT=ka[:, 128 * t:128 * (t + 1)],
                    rhs=qa[:, 512 * n:512 * (n + 1)],
                    start=True, stop=True,
                )
            pt_t = ptp.tile([128, S], BF16, name="pt", tag="pt")
            nc.scalar.activation(out=pt_t[:, :], in_=pc[:, :],
                                 func=AF.Exp, bias=ckkT[:, t:t + 1], scale=ESC)
            pts.append(pt_t)
        return pts

    def stage_b(h, pts):
        """ctx + normalize for head h."""
        for s in range(8):
            px = psum_small()
            for t in range(8):
                nc.tensor.matmul(
                    out=px[:, 0:D + 1],
                    lhsT=pts[t][:, 128 * s:128 * (s + 1)],
                    rhs=vsb[t][:, (D + 1) * h:(D + 1) * (h + 1)],
                    start=(t == 0), stop=(t == 7),
                )
            rec = work.tile([128, 1], F32, name="rec", tag="rec")
            nc.vector.reciprocal(rec[:, :], px[:, D:D + 1])
            nc.vector.tensor_scalar_mul(ctxn[s][:, D * h:D * (h + 1)],
                                        px[:, 0:D], rec[:, 0:1])

    prev = stage_a(0)
    for h in range(HPC):
        nxt = stage_a(h + 1) if h + 1 < HPC else None
        stage_b(h, prev)
        prev = nxt

    # ---------------- transpose ctxn -> ctxT (384, S) ----------------
    for s in range(8):
        for j in range(3):
            pt_ps = psum_bf()
            nc.tensor.transpose(pt_ps[:, 0:128], ctxn[s][:, 128 * j:128 * (j + 1)],
                                ident[:, :])
            cp(ctxT[j][:, 128 * s:128 * (s + 1)], pt_ps[:, 0:128])

    # ---------------- out projection + bo/2 + store -------------
    for s in range(8):
        po = psum()
        for n0, nw in ((0, 512), (512, 256)):
            for kc in range(3):
                nc.tensor.matmul(
                    out=po[:, n0:n0 + nw],
                    lhsT=ctxT[kc][:, 128 * s:128 * (s + 1)],
                    rhs=wo[kc][:, n0:n0 + nw],
                    start=(kc == 0), stop=(kc == 2),
                )
        osb = work.tile([128, E], F32, name="osb", tag="osb", bufs=2)
        nc.vector.scalar_tensor_tensor(
            out=osb[:, :], in0=po[:, 0:E], scalar=0.0,
            in1=bo_bc[:, :], op0=mybir.AluOpType.bypass, op1=mybir.AluOpType.add,
        )
        dma(out=outd[128 * s:128 * (s + 1), :], in_=osb[:, :])

    stack.close()


_NC_CACHE = None


def _build():
    global _NC_CACHE
    if _NC_CACHE is None:
        nc = bacc.Bacc("TRN2", target_bir_lowering=False, debug=False,
                       enable_asserts=True, num_devices=NCORES)
        with tile.TileContext(nc) as tc:
            _emit(tc)
        nc.compile()
        _NC_CACHE = nc
    return _NC_CACHE


def kernel(hidden_states, attention_mask, Wq, bq, Wk, bk, Wv, bv, Wo, bo, A,
           **_ignored):
    global LAST_RESULTS
    hidden_states = np.asarray(hidden_states, np.float32)
    attention_mask = np.asarray(attention_mask, np.float32)
    Wq, bq = np.asarray(Wq, np.float32), np.asarray(bq, np.float32)
    Wk, bk = np.asarray(Wk, np.float32), np.asarray(bk, np.float32)
    Wv, bv = np.asarray(Wv, np.float32), np.asarray(bv, np.float32)
    Wo, bo = np.asarray(Wo, np.float32), np.asarray(bo, np.float32)
    A = np.asarray(A, np.float32)

    B = hidden_states.shape[0]
    nc = _build()

    bf = ml_dtypes.bfloat16
    ones1 = np.ones((1, S), np.float32)
    in_maps = []
    for c in range(NCORES):
        b = c // 2
        h0 = HPC * (c % 2)
        sl = slice(h0 * D, (h0 + HPC) * D)
        hTa = np.concatenate([hidden_states[b].T, ones1], 0)
        Wqr = np.concatenate([Wq[sl], bq[sl][:, None]], 1)        # (384, 769)
        Wkr = np.concatenate([Wk[sl], bk[sl][:, None]], 1)
        WvTa = np.concatenate([Wv[sl].T, bv[sl][None, :]], 0)     # (769, 384)
        WoT = Wo[:, sl].T.copy()                                  # (384, 768)
        Apack = np.zeros((D, WEFFW), np.float32)
        for h in range(HPC):
            Apack[:, 32 * h:32 * h + R] = A[h0 + h]
        maskT = attention_mask[b, 0, 0].reshape(S // 128, 128).T
        in_maps.append({
            "hTa": np.ascontiguousarray(hTa.astype(bf)),
            "Wqr": np.ascontiguousarray(Wqr.astype(bf)),
            "Wkr": np.ascontiguousarray(Wkr.astype(bf)),
            "WvTa": np.ascontiguousarray(WvTa.astype(bf)),
            "WoT": np.ascontiguousarray(WoT.astype(bf)),
            "Apack": np.ascontiguousarray(Apack.astype(bf)),
            "maskT": np.ascontiguousarray(maskT),
            "bo2": np.ascontiguousarray((bo / 2.0)[None, :]),
        })

    res = run_bass_kernel_spmd(nc, in_maps, list(range(NCORES)),
                               trace=bool(os.environ.get("KERNEL_TRACE")))
    LAST_RESULTS = res
    parts = [res.results[c]["outp"] for c in range(NCORES)]
    out = np.stack([parts[2 * b] + parts[2 * b + 1] for b in range(B)], 0)
    return np.ascontiguousarray(out.astype(np.float32))


# revision 25
# speedup vs baseline: 1.0749x; 1.0749x over previous
"""Trainium2 Bass kernel for nn_CurvedMultiHeadAttention (B=4, S=1024, E=768, H=12, D=64, R=16).

Sharding: 8 cores; core c handles batch b=c//2 and heads h0=6*(c%2) .. h0+5
(head-parallel within a batch element). Each core computes a partial output
(its 6 heads' out-projection contribution, with bo/2 folded in); the host sums
the two partials per batch element (the unshard step for head sharding).

Math restructuring (validated against the reference at ~1e-6 rel err):
 - softmax over keys is invariant to per-query additive shifts => the qq term
   of the Mahalanobis distance drops entirely.
 - the EPS*I part of G_h contributes <1e-5 relative to scores => dropped.
 - scoresT[t,s] = sum_r kAT[r,t]*qAT[r,s];  per-key bias ckk[t] = -SCALE*kk[t]
   + mask[t] is applied as the ScalarE activation bias during exp (scoresT has
   keys on partitions, so the bias axis is the partition axis).
 - qA = (hidden @ Wq^T + bq) @ A is computed as hidden @ (Wq^T A) + bq A:
   Weff = A^T Wq is built on-device with tiny matmuls, so the big projection
   emits 16 (padded to 32) output dims per head instead of 64.
 - softmax denominator comes free as a ones column appended to v in the ctx
   matmul; bo/2 is added via a GpSimd-broadcast row during PSUM evacuation.

All heavy matmuls are bf16 with fp32 PSUM accumulation (measured end-to-end
rel err ~2.4e-3). The score/exp stage of head h+1 is software-pipelined with
the ctx stage of head h so ScalarE (exp) and TensorE overlap.
"""

import os
import numpy as np
import ml_dtypes

import concourse.bass as bass
import concourse.tile as tile
from concourse import bacc
from concourse import mybir
from concourse.bass_utils import run_bass_kernel_spmd
from concourse.masks import make_identity

F32 = mybir.dt.float32
BF16 = mybir.dt.bfloat16
AF = mybir.ActivationFunctionType

S = 1024          # sequence length
E = 768           # embed
D = 64            # head dim
R = 16            # rank
HPC = 6           # heads per core
NCORES = 8
SCALE = 1.0 / 8.0
ESC = 2.0 * SCALE  # exp scale

EAUG = E + 1            # 769 (ones row folds biases into the projections)
KCH = [128] * 6 + [1]   # contraction chunking of EAUG
WEFFW = 32 * HPC        # Weff columns, 32 per head (16 real + 16 pad)

LAST_RESULTS = None     # BassKernelResults of the most recent run (for test.py)


def _emit(tc):
    nc = tc.nc
    hTa = nc.dram_tensor("hTa", [EAUG, S], BF16, kind="ExternalInput")
    wqr = nc.dram_tensor("Wqr", [HPC * D, EAUG], BF16, kind="ExternalInput")
    wkr = nc.dram_tensor("Wkr", [HPC * D, EAUG], BF16, kind="ExternalInput")
    wvd = nc.dram_tensor("WvTa", [EAUG, HPC * D], BF16, kind="ExternalInput")
    wod = nc.dram_tensor("WoT", [HPC * D, E], BF16, kind="ExternalInput")
    apd = nc.dram_tensor("Apack", [D, WEFFW], BF16, kind="ExternalInput")
    mkd = nc.dram_tensor("maskT", [128, S // 128], F32, kind="ExternalInput")
    bod = nc.dram_tensor("bo2", [1, E], F32, kind="ExternalInput")
    outd = nc.dram_tensor("outp", [S, E], F32, kind="ExternalOutput")

    import contextlib
    stack = contextlib.ExitStack()
    const = stack.enter_context(tc.tile_pool(name="const", bufs=1))
    work = stack.enter_context(tc.tile_pool(name="work", bufs=4))
    ptp = stack.enter_context(tc.tile_pool(name="ptp", bufs=16))
    psp = stack.enter_context(tc.tile_pool(name="psp", bufs=3, space="PSUM"))

    def psum():
        return psp.tile([128, 1024], F32, name="ps", tag="ps")

    def psum_bf():
        return psp.tile([128, 1024], BF16, name="pst", tag="pst", bufs=2)

    dma = nc.sync.dma_start
    cp = nc.vector.tensor_copy

    # ---------------- constant / weight loads ----------------
    hT, wv = [], []
    for i, kc in enumerate(KCH):
        r0 = 128 * i
        hT.append(const.tile([kc, S], BF16, name=f"hT{i}", tag=f"hT{i}"))
        dma(out=hT[i][:, :], in_=hTa[r0:r0 + kc, :])
        wv.append(const.tile([kc, HPC * D], BF16, name=f"wv{i}", tag=f"wv{i}"))
        dma(out=wv[i][:, :], in_=wvd[r0:r0 + kc, :])
    wqh, wkh = [], []
    for h in range(HPC):
        wqh.append(const.tile([D, EAUG], BF16, name=f"wqh{h}", tag=f"wqh{h}"))
        dma(out=wqh[h][:, :], in_=wqr[D * h:D * (h + 1), :])
        wkh.append(const.tile([D, EAUG], BF16, name=f"wkh{h}", tag=f"wkh{h}"))
        dma(out=wkh[h][:, :], in_=wkr[D * h:D * (h + 1), :])
    wo = []
    for i in range(3):
        wo.append(const.tile([128, E], BF16, name=f"wo{i}", tag=f"wo{i}"))
        dma(out=wo[i][:, :], in_=wod[128 * i:128 * (i + 1), :])
    apk = const.tile([D, WEFFW], BF16, name="apk", tag="apk")
    dma(out=apk[:, :], in_=apd[:, :])
    maskT = const.tile([128, S // 128], F32, name="maskT", tag="maskT")
    dma(out=maskT[:, :], in_=mkd[:, :])
    bo2 = const.tile([1, E], F32, name="bo2", tag="bo2")
    dma(out=bo2[:, :], in_=bod[:, :])
    bo_bc = const.tile([128, E], F32, name="bo_bc", tag="bo_bc")
    nc.gpsimd.partition_broadcast(bo_bc[:, :], bo2[:, :])

    ones16 = const.tile([R, 1], BF16, name="ones16", tag="ones16")
    nc.vector.memset(ones16[:, :], 1.0)
    ident = const.tile([128, 128], BF16, name="ident", tag="ident")
    make_identity(nc, ident[:, :])

    vsb = [const.tile([128, HPC * (D + 1)], BF16, name=f"v{t}", tag=f"v{t}") for t in range(8)]
    ctxn = [const.tile([128, HPC * D], BF16, name=f"ctxn{s}", tag=f"ctxn{s}") for s in range(8)]
    ctxT = [const.tile([128, S], BF16, name=f"ctxT{j}", tag=f"ctxT{j}") for j in range(3)]

    # ---------------- v projection -> vsb (bf16, ones col interleaved) --------
    for t in range(8):
        pv = psum()
        for k in range(7):
            nc.tensor.matmul(
                out=pv[:, 0:HPC * D],
                lhsT=hT[k][:, 128 * t:128 * (t + 1)],
                rhs=wv[k][:, :],
                start=(k == 0), stop=(k == 6),
            )
        vst = work.tile([128, HPC * D], BF16, name="vst", tag="vst", bufs=2)
        cp(vst[:, :], pv[:, 0:HPC * D])
        vv = vsb[t][:, :].rearrange("p (h c) -> p h c", h=HPC)   # (128, 6, 65)
        cp(vv[:, :, 0:D], vst[:, :].rearrange("p (h c) -> p h c", h=HPC))
        nc.vector.memset(vv[:, :, D:D + 1], 1.0)

    # ---------------- Weff = [A^T Wq ; A^T bq] on device ----------------
    # weff{q,k}[ec] : (128|1, 192) bf16, rows = e (769 total), col 32h+r
    weff = {}
    for key, wh in (("q", wqh), ("k", wkh)):
        tiles = []
        for ec, kc in enumerate(KCH):
            pw = psum()
            for h in range(HPC):
                nc.tensor.matmul(
                    out=pw[0:kc, 32 * h:32 * h + R],
                    lhsT=wh[h][:, 128 * ec:128 * ec + kc],
                    rhs=apk[:, 32 * h:32 * h + R],
                    start=True, stop=True,
                )
            wt = const.tile([kc, WEFFW], BF16, name=f"weff{key}{ec}",
                            tag=f"weff{key}{ec}")
            cp(wt[:, :], pw[0:kc, 0:WEFFW])
            tiles.append(wt)
        weff[key] = tiles

    # ---------------- qAT/kAT for all heads: Weff.T @ hTa ----------------
    # two partition groups: heads 0-3 (cols 0:128) and heads 4-5 (cols 128:192)
    qk = {"q": [], "k": []}
    for key in ("q", "k"):
        for mt, mp in ((0, 128), (1, 64)):
            pq = psum()
            for n in range(2):
                for k in range(7):
                    nc.tensor.matmul(
                        out=pq[0:mp, 512 * n:512 * (n + 1)],
                        lhsT=weff[key][k][:, 128 * mt:128 * mt + mp],
                        rhs=hT[k][:, 512 * n:512 * (n + 1)],
                        start=(k == 0), stop=(k == 6),
                    )
            big = work.tile([128, S], BF16, name=f"{key}all{mt}", tag=f"{key}all", bufs=2)
            cp(big[0:mp, :], pq[0:mp, :])
            # per-head base-0 slices via fast bf16->bf16 copies
            for hh in range(4 if mt == 0 else 2):
                th = work.tile([R, S], BF16, name=f"{key}a", tag=f"{key}a", bufs=7)
                cp(th[:, :], big[32 * hh:32 * hh + R, :])
                qk[key].append(th)

    # ---------------- per-head attention, software-pipelined ----------------
    def stage_a(h):
        """scores + exp for head h; returns the 8 PT tiles."""
        qa, ka = qk["q"][h], qk["k"][h]
        ksq = work.tile([R, S], BF16, name="ksq", tag="ksq", bufs=2)
        nc.vector.tensor_mul(ksq[:, :], ka[:, :], ka[:, :])
        pk = psum()
        for t in range(8):
            nc.tensor.matmul(
                out=pk[:, t:t + 1],
                lhsT=ksq[:, 128 * t:128 * (t + 1)],
                rhs=ones16[:, :],
                start=True, stop=True,
            )
        ckkT = work.tile([128, S // 128], F32, name="ckkT", tag="ckkT", bufs=2)
        nc.vector.scalar_tensor_tensor(
            out=ckkT[:, :], in0=pk[:, 0:S // 128], scalar=-SCALE,
            in1=maskT[:, :], op0=mybir.AluOpType.mult, op1=mybir.AluOpType.add,
        )
        pts = []
        for t in range(8):
            pc = psum()
            for n in range(2):
                nc.tensor.matmul(
                    out=pc[:, 512 * n:512 * (n + 1)],
                    lhsT=ka[:, 128 * t:128 * (t + 1)],
                    rhs=qa[:, 512 * n:512 * (n + 1)],
                    start=True, stop=True,
                )
            pt_t = ptp.tile([128, S], BF16, name="pt", tag="pt")
            nc.scalar.activation(out=pt_t[:, :], in_=pc[:, :],
                                 func=AF.Exp, bias=ckkT[:, t:t + 1], scale=ESC)
            pts.append(pt_t)
        return pts

    def stage_b(h, pts):
        """ctx + normalize for head h."""
        for s in range(8):
            px = psum()
            for t in range(8):
                nc.tensor.matmul(
                    out=px[:, 0:D + 1],
                    lhsT=pts[t][:, 128 * s:128 * (s + 1)],
                    rhs=vsb[t][:, (D + 1) * h:(D + 1) * (h + 1)],
                    start=(t == 0), stop=(t == 7),
                )
            rec = work.tile([128, 1], F32, name="rec", tag="rec")
            nc.vector.reciprocal(rec[:, :], px[:, D:D + 1])
            nc.vector.tensor_scalar_mul(ctxn[s][:, D * h:D * (h + 1)],
                                        px[:, 0:D], rec[:, 0:1])

    prev = stage_a(0)
    for h in range(HPC):
        nxt = stage_a(h + 1) if h + 1 < HPC else None
        stage_b(h, prev)
        prev = nxt

    # ---------------- transpose ctxn -> ctxT (384, S) ----------------
    for s in range(8):
        for j in range(3):
            pt_ps = psum_bf()
            nc.tensor.transpose(pt_ps[:, 0:128], ctxn[s][:, 128 * j:128 * (j + 1)],
                                ident[:, :])
            cp(ctxT[j][:, 128 * s:128 * (s + 1)], pt_ps[:, 0:128])

    # ---------------- out projection + bo/2 + store -------------
    for s in range(8):
        po = psum()
        for n0, nw in ((0, 512), (512, 256)):
            for kc in range(3):
                nc.tensor.matmul(
                    out=po[:, n0:n0 + nw],
                    lhsT=ctxT[kc][:, 128 * s:128 * (s + 1)],
                    rhs=wo[kc][:, n0:n0 + nw],
                    start=(kc == 0), stop=(kc == 2),
                )
        osb = work.tile([128, E], F32, name="osb", tag="osb", bufs=2)
        nc.vector.scalar_tensor_tensor(
            out=osb[:, :], in0=po[:, 0:E], scalar=0.0,
            in1=bo_bc[:, :], op0=mybir.AluOpType.bypass, op1=mybir.AluOpType.add,
        )
        dma(out=outd[128 * s:128 * (s + 1), :], in_=osb[:, :])

    stack.close()


_NC_CACHE = None


def _build():
    global _NC_CACHE
    if _NC_CACHE is None:
        nc = bacc.Bacc("TRN2", target_bir_lowering=False, debug=False,
                       enable_asserts=True, num_devices=NCORES)
        with tile.TileContext(nc) as tc:
            _emit(tc)
        nc.compile()
        _NC_CACHE = nc
    return _NC_CACHE


def kernel(hidden_states, attention_mask, Wq, bq, Wk, bk, Wv, bv, Wo, bo, A,
           **_ignored):
    global LAST_RESULTS
    hidden_states = np.asarray(hidden_states, np.float32)
    attention_mask = np.asarray(attention_mask, np.float32)
    Wq, bq = np.asarray(Wq, np.float32), np.asarray(bq, np.float32)
    Wk, bk = np.asarray(Wk, np.float32), np.asarray(bk, np.float32)
    Wv, bv = np.asarray(Wv, np.float32), np.asarray(bv, np.float32)
    Wo, bo = np.asarray(Wo, np.float32), np.asarray(bo, np.float32)
    A = np.asarray(A, np.float32)

    B = hidden_states.shape[0]
    nc = _build()

    bf = ml_dtypes.bfloat16
    ones1 = np.ones((1, S), np.float32)
    in_maps = []
    for c in range(NCORES):
        b = c // 2
        h0 = HPC * (c % 2)
        sl = slice(h0 * D, (h0 + HPC) * D)
        hTa = np.concatenate([hidden_states[b].T, ones1], 0)
        Wqr = np.concatenate([Wq[sl], bq[sl][:, None]], 1)        # (384, 769)
        Wkr = np.concatenate([Wk[sl], bk[sl][:, None]], 1)
        WvTa = np.concatenate([Wv[sl].T, bv[sl][None, :]], 0)     # (769, 384)
        WoT = Wo[:, sl].T.copy()                                  # (384, 768)
        Apack = np.zeros((D, WEFFW), np.float32)
        for h in range(HPC):
            Apack[:, 32 * h:32 * h + R] = A[h0 + h]
        maskT = attention_mask[b, 0, 0].reshape(S // 128, 128).T
        in_maps.append({
            "hTa": np.ascontiguousarray(hTa.astype(bf)),
            "Wqr": np.ascontiguousarray(Wqr.astype(bf)),
            "Wkr": np.ascontiguousarray(Wkr.astype(bf)),
            "WvTa": np.ascontiguousarray(WvTa.astype(bf)),
            "WoT": np.ascontiguousarray(WoT.astype(bf)),
            "Apack": np.ascontiguousarray(Apack.astype(bf)),
            "maskT": np.ascontiguousarray(maskT),
            "bo2": np.ascontiguousarray((bo / 2.0)[None, :]),
        })

    res = run_bass_kernel_spmd(nc, in_maps, list(range(NCORES)),
                               trace=bool(os.environ.get("KERNEL_TRACE")))
    LAST_RESULTS = res
    parts = [res.results[c]["outp"] for c in range(NCORES)]
    out = np.stack([parts[2 * b] + parts[2 * b + 1] for b in range(B)], 0)
    return np.ascontiguousarray(out.astype(np.float32))
